# revision 42
# baseline (speedup 1.0000x reference)
"""LMU kernel for Trainium2, 8-core data-parallel.

Math (per batch b, with x[b] in [D, L] layout):
  u[b]    = relu(W_u @ x[b] + b_u)                              [1, L]
  m[b]    = H @ Toep(u[b])        (causal conv via Toeplitz)    [D, L]
  h[b]    = relu(W_h[:, :D] @ m[b] + W_h[:, D:] @ x[b] + b_h)   [D, L]
  y[b]    = BN(conv_w @ h[b] + conv_b)                          [D, L]

Device-side folds (host precomputes, O(params) only):
  F      = (W_h[:, :D] @ H).T, row-flipped  -> single K=128 contraction
           against the (flipped) Toeplitz of u
  C'     = (inv * conv_w).T, bias' = (conv_b - mean) * inv + beta   (BN fold)

All matmul operands are bfloat16 (weights quantized host-side, x cast
on the DVE after an f32 DMA): the PE streams 1 col/cycle either way,
but bf16 enables Fast Weight Load, which hides LDWEIGHTS entirely and
puts the dense phase at the 216 ns/matmul streaming floor.  PSUM
accumulation and the output stay fp32 (rel err ~3e-3).  Weights are
prepacked j-major so each staging DMA is one contiguous read, issued
on a single deadline-ordered sync queue (concurrent queues share the
16 SDMA engines and only delay whichever transfer is needed first).
x is prefetched three blocks ahead; each block's u -> DRAM -> Toeplitz
round-trip (~2.7us HBM write-receipt per hop) is issued a full block
early so only block 0 pays it.  Batch dim sharded 8 ways.
"""

import os
import numpy as np

import concourse.bass as bass
import concourse.mybir as mybir
from concourse import bacc
from concourse.tile import TileContext
from concourse.bass_utils import run_bass_kernel_spmd

B, D, L = 256, 768, 128
NCORES = 8
BPC = B // NCORES          # batches per core
NB = 4                     # batches per column block
NCB = BPC // NB            # column blocks per core
NCOL = NB * L              # 512 columns per block
KC = D // 128              # 6 chunks of 128 over the D dim
THETA = 128.0
BN_EPS = 1e-5

TRACE = False
LAST_EXEC_NS = None

_H_CACHE = None
_NC_CACHE = None


def _impulse_response():
    """Replicates the reference's H = impulse response [D, L], on CPU."""
    global _H_CACHE
    if _H_CACHE is not None:
        return _H_CACHE
    import jax
    import jax.numpy as jnp
    from jax.scipy.linalg import expm

    cpu = jax.devices("cpu")[0]
    with jax.default_device(cpu):
        Q = np.arange(D, dtype=np.float32)
        R = ((2.0 * Q + 1.0) / THETA)[:, None]
        i, j = np.meshgrid(Q, Q, indexing="ij")
        A = (np.where(i < j, -1.0, (-1.0) ** (i - j + 1)).astype(np.float32)) * R
        Bm = (((-1.0) ** Q)[:, None]).astype(np.float32) * R
        Maug = np.zeros((D + 1, D + 1), dtype=np.float32)
        Maug[:D, :D] = A
        Maug[:D, D:] = Bm
        E = expm(jnp.asarray(Maug))
        Ad = E[:D, :D]
        Bd = E[:D, D:]

        def step(Apow, _):
            return Ad @ Apow, (Apow @ Bd)[:, 0]

        _, H = jax.lax.scan(step, jnp.eye(D, dtype=jnp.float32), None, length=L)
        _H_CACHE = np.asarray(H).T.astype(np.float32)  # [D, L]
    return _H_CACHE


def _build_nc():
    """Builds the (static) 8-core SPMD Bass program."""
    f32 = mybir.dt.float32
    f32r = mybir.dt.float32r
    bf16 = mybir.dt.bfloat16
    nc = bacc.Bacc("TRN2", target_bir_lowering=False, debug=False, num_devices=NCORES)

    x_d = nc.dram_tensor("x", [BPC, D, L], f32, kind="ExternalInput").ap()
    # j-major packed weights: [j, i*128+p, c] so each output-chunk stage is
    # one fully contiguous read (SDMA merges descriptors -> fast receipt)
    whxT_d = nc.dram_tensor("whxT", [KC, D, 128], bf16, kind="ExternalInput").ap()
    ct_d = nc.dram_tensor("ct", [KC, D, 128], bf16, kind="ExternalInput").ap()
    f_d = nc.dram_tensor("fmat", [L, D], bf16, kind="ExternalInput").ap()
    wu_d = nc.dram_tensor("wu", [128, KC], bf16, kind="ExternalInput").ap()
    vecs_d = nc.dram_tensor("vecs", [D, 3], f32, kind="ExternalInput").ap()
    out_d = nc.dram_tensor("out", [BPC, D, L], f32r, kind="ExternalOutput").ap()
    upad_d = nc.dram_tensor("upad", [BPC * 2 * L], bf16).ap()  # internal scratch

    XSTR_B, XSTR_D = D * L, L  # element strides of x / out in DRAM
    Relu = mybir.ActivationFunctionType.Relu

    with TileContext(nc) as tc:
        with (
            tc.tile_pool(name="const", bufs=1) as const,
            tc.tile_pool(name="xpool", bufs=14) as xpool,
            tc.tile_pool(name="xbpool", bufs=24) as xbpool,
            tc.tile_pool(name="hpool", bufs=12) as hpool,
            tc.tile_pool(name="tpool", bufs=4) as tpool,
            tc.tile_pool(name="opool", bufs=6) as opool,
            tc.tile_pool(name="upool", bufs=2) as upool,
            tc.tile_pool(name="pu", bufs=2, space="PSUM") as pu,
            tc.tile_pool(name="p3", bufs=3, space="PSUM") as p3,
            tc.tile_pool(name="p4", bufs=3, space="PSUM") as p4,
        ):
            # ---- constant tiles (DMA'd directly, no staging casts) ----
            whx_r = const.tile([128, KC, D], bf16)   # [d part | i_chunk | d' col]
            ct_r = const.tile([128, KC, D], bf16)    # [dh part | i_chunk | o col]
            f_r = const.tile([128, D], bf16)         # [t' part | d]
            wu_r = const.tile([128, KC], bf16)
            vecs_sb = const.tile([128, KC, 3], f32)  # b_h, bias', b_u
            zt = const.tile([128, 2 * BPC], bf16)

            # wu first on sync (ahead of x0): it gates the very first matmul
            # and would otherwise sit behind scalar's ACT table load
            nc.sync.dma_start(out=wu_r[:], in_=wu_d)
            nc.scalar.dma_start(
                out=vecs_sb[:],
                in_=bass.AP(tensor=vecs_d.tensor, offset=0,
                            ap=[[3, 128], [384, KC], [1, 3]]),
            )
            # zero the upad scratch (pad halves stay zero forever)
            nc.vector.memset(zt[:], 0.0)
            nc.gpsimd.dma_start(
                out=bass.AP(tensor=upad_d.tensor, offset=0,
                            ap=[[1, BPC * 2 * L]]),
                in_=zt[:],
            )

            def load_x(cb, casting=False):
                """DMA x tiles for column block cb into bf16 tiles.

                Early blocks: sync HWDGE load + Vector cast (low latency).
                Steady state (casting=True): gpsimd SWDGE casting DMA
                converts f32->bf16 in the datapath -- no staging tile, no
                Vector work.
                """
                b0 = cb * NB
                xr = []
                for i in range(KC):
                    xb = xbpool.tile([128, NCOL], bf16, tag="xb")
                    src_ap = bass.AP(
                        tensor=x_d.tensor,
                        offset=b0 * XSTR_B + i * 128 * XSTR_D,
                        ap=[[XSTR_D, 128], [XSTR_B, NB], [1, L]],
                    )
                    if casting:
                        nc.gpsimd.dma_start(out=xb[:], in_=src_ap)
                    else:
                        xt = xpool.tile([128, NCOL], f32, tag="xt")
                        nc.sync.dma_start(out=xt[:], in_=src_ap)
                        nc.vector.tensor_copy(xb[:], xt[:])
                    xr.append(xb)
                return xr

            def stage_w_chunk(dram, dst, j, eng):
                """Stage output-chunk j of a packed [KC, D, 128] weight."""
                eng.dma_start(
                    out=dst[:, :, j * 128:(j + 1) * 128],
                    in_=bass.AP(
                        tensor=dram.tensor,
                        offset=j * D * 128,
                        ap=[[128, 128], [128 * 128, KC], [1, 128]],
                    ),
                )

            def compute_u(cb, xr, chain_eng):
                """u = relu(W_u @ x + b_u) -> upad scratch -> Toeplitz tile."""
                psu = pu.tile([1, NCOL], f32, tag="pu")
                for i in range(KC):
                    nc.tensor.matmul(psu[:], wu_r[:, i:i + 1], xr[i][:],
                                     start=(i == 0), stop=(i == KC - 1))
                u_sb = upool.tile([1, NCOL], bf16, tag="u")
                nc.scalar.activation(u_sb[:], psu[:], Relu,
                                     bias=vecs_sb[0:1, 0, 2:3])
                t_r = tpool.tile([128, NCOL], bf16, tag="tr")
                if cb == 0:
                    # latency-critical first chain: per-batch write/read
                    # pairs whose completion receipts all overlap
                    for b in range(NB):
                        chain_eng.dma_start(
                            out=bass.AP(tensor=upad_d.tensor,
                                        offset=b * 2 * L + L,
                                        ap=[[1, L]]),
                            in_=u_sb[:, b * L:(b + 1) * L],
                        )
                    for b in range(NB):
                        chain_eng.dma_start(
                            out=t_r[:, b * L:(b + 1) * L],
                            in_=bass.AP(tensor=upad_d.tensor,
                                        offset=b * 2 * L + 1,
                                        ap=[[1, 128], [1, L]]),
                        )
                    return t_r
                chain_eng.dma_start(
                    out=bass.AP(tensor=upad_d.tensor,
                                offset=cb * NB * 2 * L + L,
                                ap=[[2 * L, NB], [1, L]]),
                    in_=u_sb[:],
                )
                chain_eng.dma_start(
                    out=t_r[:],
                    in_=bass.AP(tensor=upad_d.tensor,
                                offset=cb * NB * 2 * L + 1,
                                ap=[[1, 128], [2 * L, NB], [1, L]]),
                )
                return t_r

            def step3(cb, xr, t_r, js, hs):
                for j in js:
                    ps3 = p3.tile([128, NCOL], f32, tag="ps3")
                    for i in range(KC):
                        nc.tensor.matmul(ps3[:], whx_r[:, i, j * 128:(j + 1) * 128],
                                         xr[i][:], start=(i == 0), stop=False)
                    nc.tensor.matmul(ps3[:], f_r[:, j * 128:(j + 1) * 128], t_r[:],
                                     start=False, stop=True)
                    hj = hpool.tile([128, NCOL], bf16, tag="h")
                    nc.scalar.activation(hj[:], ps3[:], Relu,
                                         bias=vecs_sb[:, j, 0:1])
                    hs.append(hj)

            def step4(cb, hs):
                b0 = cb * NB
                for j in range(KC):
                    ps4 = p4.tile([128, NCOL], f32, tag="ps4")
                    for i in range(KC):
                        nc.tensor.matmul(ps4[:], ct_r[:, i, j * 128:(j + 1) * 128],
                                         hs[i][:], start=(i == 0), stop=(i == KC - 1))
                    oj = opool.tile([128, NCOL], f32r, tag="o")
                    nc.vector.tensor_scalar_add(oj[:], ps4[:], vecs_sb[:, j, 1:2])
                    nc.scalar.dma_start(
                        out=bass.AP(
                            tensor=out_d.tensor,
                            offset=b0 * XSTR_B + j * 128 * XSTR_D,
                            ap=[[XSTR_D, 128], [XSTR_B, NB], [1, L]],
                        ),
                        in_=oj[:],
                    )

            # ---- prologue: ONE deadline-ordered staging queue (sync).
            # Concurrent queues share the same 16 SDMA engines, so a second
            # queue only delays whichever transfer is needed first — strict
            # need-order on one queue wins.  Deadlines (PE time): x(0) asap,
            # whx j0-2 ~17us (first step3 matmuls), f ~22 (first Toeplitz
            # matmul), x(1) ~22 (u(1) fills the Toeplitz-wait), whx j3-5
            # ~26, ct ~30 (step4(0)), x(2) ~36, x(3+) one block ahead.
            xr = {0: load_x(0)}
            for j in range(3):
                stage_w_chunk(whxT_d, whx_r, j, nc.sync)
            nc.sync.dma_start(out=f_r[:], in_=f_d)
            xr[1] = load_x(1)
            t = {0: compute_u(0, xr[0], nc.scalar)}
            for j in range(3, KC):
                stage_w_chunk(whxT_d, whx_r, j, nc.sync)
            for j in range(KC):
                stage_w_chunk(ct_d, ct_r, j, nc.sync)
            xr[2] = load_x(2)

            # ---- software-pipelined main loop: x stays three blocks ahead;
            # the u->Toeplitz chain for block cb+1 is issued at the top of
            # block cb (a full block of step3/step4 hides its ~8us DMA
            # round-trip latency).
            for cb in range(NCB):
                if cb + 3 < NCB:
                    xr[cb + 3] = load_x(cb + 3)
                if cb >= 1 and cb + 2 < NCB:
                    t[cb + 2] = compute_u(cb + 2, xr[cb + 2], nc.gpsimd)
                hs = []
                if cb == 0:
                    # Block 0 is latency-bound on the first Toeplitz
                    # round-trip (~13us).  Open the first three ps3 groups
                    # with their x-matmuls, issue u(1)/u(2) so the PE has
                    # independent work during the wait, and hint the
                    # scheduler (wait_until) to place the t-closes last.
                    ps3s = []
                    for j in range(3):
                        ps3 = p3.tile([128, NCOL], f32, tag="ps3")
                        for i in range(KC):
                            nc.tensor.matmul(
                                ps3[:], whx_r[:, i, j * 128:(j + 1) * 128],
                                xr[0][i][:], start=(i == 0), stop=False)
                        ps3s.append(ps3)
                    t[1] = compute_u(1, xr[1], nc.scalar)
                    t[2] = compute_u(2, xr[2], nc.gpsimd)
                    for j in range(3):
                        nc.tensor.matmul(
                            ps3s[j][:], f_r[:, j * 128:(j + 1) * 128],
                            t[0][:], start=False, stop=True)
                        hj = hpool.tile([128, NCOL], bf16, tag="h")
                        nc.scalar.activation(hj[:], ps3s[j][:], Relu,
                                             bias=vecs_sb[:, j, 0:1])
                        hs.append(hj)
                    step3(cb, xr[cb], t[cb], [3, 4, 5], hs)
                else:
                    step3(cb, xr[cb], t[cb], [0, 1, 2], hs)
                    step3(cb, xr[cb], t[cb], [3, 4, 5], hs)
                step4(cb, hs)

    if not nc.is_finalized():
        nc.finalize()
    return nc


def _get_nc():
    global _NC_CACHE
    if _NC_CACHE is None:
        _NC_CACHE = _build_nc()
    return _NC_CACHE


def _ensure_ntff_hook():
    """Register the NTFF profile hook if the deployment lacks antenv.axon_hooks."""
    import sys
    import types
    try:
        from antenv.axon_hooks import get_axon_ntff_profile_hook  # noqa: F401
        return
    except ImportError:
        pass
    try:
        from trn_agent_boot.trn_boot import _ntff_profile_via_ctypes
        hook = _ntff_profile_via_ctypes("/opt/axon/libaxon_pjrt.so")
        mod = types.ModuleType("antenv.axon_hooks")
        mod.get_axon_ntff_profile_hook = lambda: hook
        mod.set_axon_ntff_profile_hook = lambda h: None
        import antenv
        sys.modules["antenv.axon_hooks"] = mod
        antenv.axon_hooks = mod
    except Exception:
        pass


def kernel(x, W_u, b_u, W_h, b_h, conv_w, conv_b, bn_gamma, bn_beta, bn_mean,
           bn_var):
    global LAST_EXEC_NS
    x = np.ascontiguousarray(np.asarray(x, dtype=np.float32))
    W_u = np.asarray(W_u, dtype=np.float64)
    b_u = np.asarray(b_u, dtype=np.float64)
    W_h = np.asarray(W_h, dtype=np.float64)
    b_h = np.asarray(b_h, dtype=np.float64)
    conv_w = np.asarray(conv_w, dtype=np.float64)
    conv_b = np.asarray(conv_b, dtype=np.float64)
    bn_gamma = np.asarray(bn_gamma, dtype=np.float64)
    bn_beta = np.asarray(bn_beta, dtype=np.float64)
    bn_mean = np.asarray(bn_mean, dtype=np.float64)
    bn_var = np.asarray(bn_var, dtype=np.float64)
    assert x.shape == (B, D, L)

    H = _impulse_response().astype(np.float64)  # [D, L]

    # host folds (O(params) only)
    F = (W_h[:, :D] @ H).T[::-1, :]                      # [L, D], row-flipped
    inv = bn_gamma / np.sqrt(bn_var + BN_EPS)
    # j-major contiguous packing: [j, K-row, out-col-within-chunk]
    whxT = np.ascontiguousarray(
        W_h[:, D:].T.reshape(D, KC, 128).transpose(1, 0, 2))       # [KC, D, 128]
    ct = np.ascontiguousarray(
        (conv_w[:, :, 0] * inv[:, None]).T.reshape(D, KC, 128).transpose(1, 0, 2))
    bias2 = (conv_b - bn_mean) * inv + bn_beta
    wu = np.ascontiguousarray(W_u[0].reshape(KC, 128).T)  # [128, KC]
    vecs = np.stack([b_h, bias2, np.full(D, b_u[0])], axis=1)  # [D, 3]

    nc = _get_nc()
    import ml_dtypes
    bf = ml_dtypes.bfloat16
    shared = {
        "whxT": whxT.astype(bf),
        "ct": ct.astype(bf),
        "fmat": np.ascontiguousarray(F).astype(bf),
        "wu": wu.astype(bf),
        "vecs": vecs.astype(np.float32),
    }
    in_maps = []
    for c in range(NCORES):
        m = dict(shared)
        m["x"] = x[c * BPC:(c + 1) * BPC]
        in_maps.append(m)

    if TRACE:
        _ensure_ntff_hook()
    res = run_bass_kernel_spmd(nc, in_maps, list(range(NCORES)), trace=TRACE)
    LAST_EXEC_NS = res.exec_time_ns
    out = np.concatenate([res.results[c]["out"] for c in range(NCORES)], axis=0)
    return out


# revision 47
# speedup vs baseline: 1.0404x; 1.0404x over previous
"""LMU kernel for Trainium2, 8-core data-parallel.

Math (per batch b, with x[b] in [D, L] layout):
  u[b]    = relu(W_u @ x[b] + b_u)                              [1, L]
  m[b]    = H @ Toep(u[b])        (causal conv via Toeplitz)    [D, L]
  h[b]    = relu(W_h[:, :D] @ m[b] + W_h[:, D:] @ x[b] + b_h)   [D, L]
  y[b]    = BN(conv_w @ h[b] + conv_b)                          [D, L]

Device-side folds (host precomputes, O(params) only):
  F      = (W_h[:, :D] @ H).T, row-flipped  -> single K=128 contraction
           against the (flipped) Toeplitz of u
  C'     = (inv * conv_w).T, bias' = (conv_b - mean) * inv + beta   (BN fold)

All matmul operands are bfloat16 (weights quantized host-side, x cast
on the DVE after an f32 DMA): the PE streams 1 col/cycle either way,
but bf16 enables Fast Weight Load, which hides LDWEIGHTS entirely and
puts the dense phase at the 216 ns/matmul streaming floor.  PSUM
accumulation and the output stay fp32 (rel err ~3e-3).  Weights are
prepacked j-major so each staging DMA is one contiguous read, issued
on a single deadline-ordered sync queue (concurrent queues share the
16 SDMA engines and only delay whichever transfer is needed first).
x is prefetched three blocks ahead; each block's u -> DRAM -> Toeplitz
round-trip (~2.7us HBM write-receipt per hop) is issued a full block
early so only block 0 pays it.  Batch dim sharded 8 ways.
"""

import os
import numpy as np

import concourse.bass as bass
import concourse.mybir as mybir
from concourse import bacc
from concourse.tile import TileContext
from concourse.bass_utils import run_bass_kernel_spmd

B, D, L = 256, 768, 128
NCORES = 8
BPC = B // NCORES          # batches per core
NB = 4                     # batches per column block
NCB = BPC // NB            # column blocks per core
NCOL = NB * L              # 512 columns per block
KC = D // 128              # 6 chunks of 128 over the D dim
THETA = 128.0
BN_EPS = 1e-5

TRACE = False
LAST_EXEC_NS = None

_H_CACHE = None
_NC_CACHE = None


def _impulse_response():
    """Replicates the reference's H = impulse response [D, L], on CPU."""
    global _H_CACHE
    if _H_CACHE is not None:
        return _H_CACHE
    import jax
    import jax.numpy as jnp
    from jax.scipy.linalg import expm

    cpu = jax.devices("cpu")[0]
    with jax.default_device(cpu):
        Q = np.arange(D, dtype=np.float32)
        R = ((2.0 * Q + 1.0) / THETA)[:, None]
        i, j = np.meshgrid(Q, Q, indexing="ij")
        A = (np.where(i < j, -1.0, (-1.0) ** (i - j + 1)).astype(np.float32)) * R
        Bm = (((-1.0) ** Q)[:, None]).astype(np.float32) * R
        Maug = np.zeros((D + 1, D + 1), dtype=np.float32)
        Maug[:D, :D] = A
        Maug[:D, D:] = Bm
        E = expm(jnp.asarray(Maug))
        Ad = E[:D, :D]
        Bd = E[:D, D:]

        def step(Apow, _):
            return Ad @ Apow, (Apow @ Bd)[:, 0]

        _, H = jax.lax.scan(step, jnp.eye(D, dtype=jnp.float32), None, length=L)
        _H_CACHE = np.asarray(H).T.astype(np.float32)  # [D, L]
    return _H_CACHE


def _build_nc():
    """Builds the (static) 8-core SPMD Bass program."""
    f32 = mybir.dt.float32
    f32r = mybir.dt.float32r
    bf16 = mybir.dt.bfloat16
    nc = bacc.Bacc("TRN2", target_bir_lowering=False, debug=False, num_devices=NCORES)

    x_d = nc.dram_tensor("x", [BPC, D, L], f32, kind="ExternalInput").ap()
    # j-major packed weights: [j, i*128+p, c] so each output-chunk stage is
    # one fully contiguous read (SDMA merges descriptors -> fast receipt)
    whxT_d = nc.dram_tensor("whxT", [KC, D, 128], bf16, kind="ExternalInput").ap()
    ct_d = nc.dram_tensor("ct", [KC, D, 128], bf16, kind="ExternalInput").ap()
    f_d = nc.dram_tensor("fmat", [L, D], bf16, kind="ExternalInput").ap()
    wu_d = nc.dram_tensor("wu", [128, KC], bf16, kind="ExternalInput").ap()
    vecs_d = nc.dram_tensor("vecs", [D, 3], f32, kind="ExternalInput").ap()
    out_d = nc.dram_tensor("out", [BPC, D, L], f32r, kind="ExternalOutput").ap()
    upad_d = nc.dram_tensor("upad", [BPC * 2 * L], bf16).ap()  # internal scratch

    XSTR_B, XSTR_D = D * L, L  # element strides of x / out in DRAM
    Relu = mybir.ActivationFunctionType.Relu

    with TileContext(nc) as tc:
        with (
            tc.tile_pool(name="const", bufs=1) as const,
            tc.tile_pool(name="xpool", bufs=14) as xpool,
            tc.tile_pool(name="xbpool", bufs=24) as xbpool,
            tc.tile_pool(name="hpool", bufs=12) as hpool,
            tc.tile_pool(name="tpool", bufs=4) as tpool,
            tc.tile_pool(name="opool", bufs=6) as opool,
            tc.tile_pool(name="upool", bufs=2) as upool,
            tc.tile_pool(name="pu", bufs=2, space="PSUM") as pu,
            tc.tile_pool(name="p3", bufs=3, space="PSUM") as p3,
            tc.tile_pool(name="p4", bufs=3, space="PSUM") as p4,
        ):
            # ---- constant tiles (DMA'd directly, no staging casts) ----
            whx_r = const.tile([128, KC, D], bf16)   # [d part | i_chunk | d' col]
            ct_r = const.tile([128, KC, D], bf16)    # [dh part | i_chunk | o col]
            f_r = const.tile([128, D], bf16)         # [t' part | d]
            wu_r = const.tile([128, KC], bf16)
            vecs_sb = const.tile([128, KC, 3], f32)  # b_h, bias', b_u
            zt = const.tile([128, 2 * BPC], bf16)

            # wu first on sync (ahead of x0): it gates the very first matmul
            # and would otherwise sit behind scalar's ACT table load
            nc.sync.dma_start(out=wu_r[:], in_=wu_d)
            nc.scalar.dma_start(
                out=vecs_sb[:],
                in_=bass.AP(tensor=vecs_d.tensor, offset=0,
                            ap=[[3, 128], [384, KC], [1, 3]]),
            )
            # zero the upad scratch (pad halves stay zero forever)
            nc.vector.memset(zt[:], 0.0)
            nc.gpsimd.dma_start(
                out=bass.AP(tensor=upad_d.tensor, offset=0,
                            ap=[[1, BPC * 2 * L]]),
                in_=zt[:],
            )

            def load_x_dma(cb):
                """Issue the f32 x-tile DMAs for column block cb (sync)."""
                b0 = cb * NB
                xts = []
                for i in range(KC):
                    xt = xpool.tile([128, NCOL], f32, tag="xt")
                    nc.sync.dma_start(
                        out=xt[:],
                        in_=bass.AP(
                            tensor=x_d.tensor,
                            offset=b0 * XSTR_B + i * 128 * XSTR_D,
                            ap=[[XSTR_D, 128], [XSTR_B, NB], [1, L]],
                        ),
                    )
                    xts.append(xt)
                return xts

            def cast_x(xts):
                """DVE-cast staged f32 tiles to bf16 matmul operands.

                Issued SEPARATELY from (and later than) the DMAs: a cast
                waiting on a late DMA receipt must not head-of-line-block
                the bias-adds behind it in the Vector FIFO.
                """
                xr = []
                for xt in xts:
                    xb = xbpool.tile([128, NCOL], bf16, tag="xb")
                    nc.vector.tensor_copy(xb[:], xt[:])
                    xr.append(xb)
                return xr

            def load_x(cb):
                return cast_x(load_x_dma(cb))

            def stage_w_chunk(dram, dst, j, eng):
                """Stage output-chunk j of a packed [KC, D, 128] weight."""
                eng.dma_start(
                    out=dst[:, :, j * 128:(j + 1) * 128],
                    in_=bass.AP(
                        tensor=dram.tensor,
                        offset=j * D * 128,
                        ap=[[128, 128], [128 * 128, KC], [1, 128]],
                    ),
                )

            def compute_u(cb, xr, chain_eng):
                """u = relu(W_u @ x + b_u) -> upad scratch -> Toeplitz tile."""
                psu = pu.tile([1, NCOL], f32, tag="pu")
                for i in range(KC):
                    nc.tensor.matmul(psu[:], wu_r[:, i:i + 1], xr[i][:],
                                     start=(i == 0), stop=(i == KC - 1))
                u_sb = upool.tile([1, NCOL], bf16, tag="u")
                nc.scalar.activation(u_sb[:], psu[:], Relu,
                                     bias=vecs_sb[0:1, 0, 2:3])
                t_r = tpool.tile([128, NCOL], bf16, tag="tr")
                if cb == 0:
                    # latency-critical first chain: per-batch write/read
                    # pairs whose completion receipts all overlap
                    for b in range(NB):
                        chain_eng.dma_start(
                            out=bass.AP(tensor=upad_d.tensor,
                                        offset=b * 2 * L + L,
                                        ap=[[1, L]]),
                            in_=u_sb[:, b * L:(b + 1) * L],
                        )
                    for b in range(NB):
                        chain_eng.dma_start(
                            out=t_r[:, b * L:(b + 1) * L],
                            in_=bass.AP(tensor=upad_d.tensor,
                                        offset=b * 2 * L + 1,
                                        ap=[[1, 128], [1, L]]),
                        )
                    return t_r
                chain_eng.dma_start(
                    out=bass.AP(tensor=upad_d.tensor,
                                offset=cb * NB * 2 * L + L,
                                ap=[[2 * L, NB], [1, L]]),
                    in_=u_sb[:],
                )
                chain_eng.dma_start(
                    out=t_r[:],
                    in_=bass.AP(tensor=upad_d.tensor,
                                offset=cb * NB * 2 * L + 1,
                                ap=[[1, 128], [2 * L, NB], [1, L]]),
                )
                return t_r

            def step3(cb, xr, t_r, js, hs):
                for j in js:
                    ps3 = p3.tile([128, NCOL], f32, tag="ps3")
                    for i in range(KC):
                        nc.tensor.matmul(ps3[:], whx_r[:, i, j * 128:(j + 1) * 128],
                                         xr[i][:], start=(i == 0), stop=False)
                    nc.tensor.matmul(ps3[:], f_r[:, j * 128:(j + 1) * 128], t_r[:],
                                     start=False, stop=True)
                    hj = hpool.tile([128, NCOL], bf16, tag="h")
                    nc.scalar.activation(hj[:], ps3[:], Relu,
                                         bias=vecs_sb[:, j, 0:1])
                    hs.append(hj)

            def step4(cb, hs):
                b0 = cb * NB
                for j in range(KC):
                    ps4 = p4.tile([128, NCOL], f32, tag="ps4")
                    for i in range(KC):
                        nc.tensor.matmul(ps4[:], ct_r[:, i, j * 128:(j + 1) * 128],
                                         hs[i][:], start=(i == 0), stop=(i == KC - 1))
                    oj = opool.tile([128, NCOL], f32r, tag="o")
                    nc.vector.tensor_scalar_add(oj[:], ps4[:], vecs_sb[:, j, 1:2])
                    nc.scalar.dma_start(
                        out=bass.AP(
                            tensor=out_d.tensor,
                            offset=b0 * XSTR_B + j * 128 * XSTR_D,
                            ap=[[XSTR_D, 128], [XSTR_B, NB], [1, L]],
                        ),
                        in_=oj[:],
                    )

            # ---- prologue: ONE deadline-ordered staging queue (sync).
            # Concurrent queues share the same 16 SDMA engines, so a second
            # queue only delays whichever transfer is needed first — strict
            # need-order on one queue wins.  Deadlines (PE time): x(0) asap,
            # whx j0-2 ~17us (first step3 matmuls), f ~22 (first Toeplitz
            # matmul), x(1) ~22 (u(1) fills the Toeplitz-wait), whx j3-5
            # ~26, ct ~30 (step4(0)), x(2) ~36, x(3+) one block ahead.
            xr = {0: load_x(0)}
            for j in range(3):
                stage_w_chunk(whxT_d, whx_r, j, nc.sync)
            nc.sync.dma_start(out=f_r[:], in_=f_d)
            xr[1] = load_x(1)
            t = {0: compute_u(0, xr[0], nc.scalar)}
            for j in range(3, KC):
                stage_w_chunk(whxT_d, whx_r, j, nc.sync)
            for j in range(KC):
                stage_w_chunk(ct_d, ct_r, j, nc.sync)
            xr[2] = load_x(2)

            # ---- software-pipelined main loop: x stays three blocks ahead;
            # the u->Toeplitz chain for block cb+1 is issued at the top of
            # block cb (a full block of step3/step4 hides its ~8us DMA
            # round-trip latency).
            xts = {}
            for cb in range(NCB):
                if cb + 3 < NCB:
                    xts[cb + 3] = load_x_dma(cb + 3)
                if cb >= 1 and cb + 2 < NCB:
                    t[cb + 2] = compute_u(cb + 2, xr[cb + 2], nc.gpsimd)
                hs = []
                if cb == 0:
                    # Block 0 is latency-bound on the first Toeplitz
                    # round-trip (~13us).  Open the first three ps3 groups
                    # with their x-matmuls, issue u(1)/u(2) so the PE has
                    # independent work during the wait, and hint the
                    # scheduler (wait_until) to place the t-closes last.
                    ps3s = []
                    for j in range(3):
                        ps3 = p3.tile([128, NCOL], f32, tag="ps3")
                        for i in range(KC):
                            nc.tensor.matmul(
                                ps3[:], whx_r[:, i, j * 128:(j + 1) * 128],
                                xr[0][i][:], start=(i == 0), stop=False)
                        ps3s.append(ps3)
                    t[1] = compute_u(1, xr[1], nc.scalar)
                    t[2] = compute_u(2, xr[2], nc.gpsimd)
                    for j in range(3):
                        nc.tensor.matmul(
                            ps3s[j][:], f_r[:, j * 128:(j + 1) * 128],
                            t[0][:], start=False, stop=True)
                        hj = hpool.tile([128, NCOL], bf16, tag="h")
                        nc.scalar.activation(hj[:], ps3s[j][:], Relu,
                                             bias=vecs_sb[:, j, 0:1])
                        hs.append(hj)
                    step3(cb, xr[cb], t[cb], [3, 4, 5], hs)
                else:
                    step3(cb, xr[cb], t[cb], [0, 1, 2], hs)
                    step3(cb, xr[cb], t[cb], [3, 4, 5], hs)
                step4(cb, hs)
                # casts issued after this block's adds: x-receipt jitter
                # can no longer stall the add->store->relu chain
                if cb + 3 < NCB:
                    xr[cb + 3] = cast_x(xts.pop(cb + 3))

    if not nc.is_finalized():
        nc.finalize()
    return nc


def _get_nc():
    global _NC_CACHE
    if _NC_CACHE is None:
        _NC_CACHE = _build_nc()
    return _NC_CACHE


def _ensure_ntff_hook():
    """Register the NTFF profile hook if the deployment lacks antenv.axon_hooks."""
    import sys
    import types
    try:
        from antenv.axon_hooks import get_axon_ntff_profile_hook  # noqa: F401
        return
    except ImportError:
        pass
    try:
        from trn_agent_boot.trn_boot import _ntff_profile_via_ctypes
        hook = _ntff_profile_via_ctypes("/opt/axon/libaxon_pjrt.so")
        mod = types.ModuleType("antenv.axon_hooks")
        mod.get_axon_ntff_profile_hook = lambda: hook
        mod.set_axon_ntff_profile_hook = lambda h: None
        import antenv
        sys.modules["antenv.axon_hooks"] = mod
        antenv.axon_hooks = mod
    except Exception:
        pass


def kernel(x, W_u, b_u, W_h, b_h, conv_w, conv_b, bn_gamma, bn_beta, bn_mean,
           bn_var):
    global LAST_EXEC_NS
    x = np.ascontiguousarray(np.asarray(x, dtype=np.float32))
    W_u = np.asarray(W_u, dtype=np.float64)
    b_u = np.asarray(b_u, dtype=np.float64)
    W_h = np.asarray(W_h, dtype=np.float64)
    b_h = np.asarray(b_h, dtype=np.float64)
    conv_w = np.asarray(conv_w, dtype=np.float64)
    conv_b = np.asarray(conv_b, dtype=np.float64)
    bn_gamma = np.asarray(bn_gamma, dtype=np.float64)
    bn_beta = np.asarray(bn_beta, dtype=np.float64)
    bn_mean = np.asarray(bn_mean, dtype=np.float64)
    bn_var = np.asarray(bn_var, dtype=np.float64)
    assert x.shape == (B, D, L)

    H = _impulse_response().astype(np.float64)  # [D, L]

    # host folds (O(params) only)
    F = (W_h[:, :D] @ H).T[::-1, :]                      # [L, D], row-flipped
    inv = bn_gamma / np.sqrt(bn_var + BN_EPS)
    # j-major contiguous packing: [j, K-row, out-col-within-chunk]
    whxT = np.ascontiguousarray(
        W_h[:, D:].T.reshape(D, KC, 128).transpose(1, 0, 2))       # [KC, D, 128]
    ct = np.ascontiguousarray(
        (conv_w[:, :, 0] * inv[:, None]).T.reshape(D, KC, 128).transpose(1, 0, 2))
    bias2 = (conv_b - bn_mean) * inv + bn_beta
    wu = np.ascontiguousarray(W_u[0].reshape(KC, 128).T)  # [128, KC]
    vecs = np.stack([b_h, bias2, np.full(D, b_u[0])], axis=1)  # [D, 3]

    nc = _get_nc()
    import ml_dtypes
    bf = ml_dtypes.bfloat16
    shared = {
        "whxT": whxT.astype(bf),
        "ct": ct.astype(bf),
        "fmat": np.ascontiguousarray(F).astype(bf),
        "wu": wu.astype(bf),
        "vecs": vecs.astype(np.float32),
    }
    in_maps = []
    for c in range(NCORES):
        m = dict(shared)
        m["x"] = x[c * BPC:(c + 1) * BPC]
        in_maps.append(m)

    if TRACE:
        _ensure_ntff_hook()
    res = run_bass_kernel_spmd(nc, in_maps, list(range(NCORES)), trace=TRACE)
    LAST_EXEC_NS = res.exec_time_ns
    out = np.concatenate([res.results[c]["out"] for c in range(NCORES)], axis=0)
    return out
